# revision 28
# baseline (speedup 1.0000x reference)
"""Multi-head causal attention block on 8 TRN2 NeuronCores.

Strategy: 8-way tensor parallel over heads (2 heads/core, both batch rows
on every core). Mixed precision: fp16 for dense1 inputs / k / q / P / V
and dense2, fp32 PSUM accumulation everywhere.

Per core:
  warmup: ~5us of junk N=64 matmuls so the PE HAM clock-gate reaches
          K=8/8 (2.4GHz) before the first real projection matmul.
  phase 1: project x -> kT/qT channel-major fp16 (heads stacked on
           partitions 0-63 / 64-127), v token-major fp16 via PE
           transpose of the channel-major result. x DMAs are issued one
           128-channel slice at a time so the first matmul starts as
           soon as 1/8 of a chunk has landed.
  phase 2: causal attention in S^T orientation (probabilities come out
           pre-transposed for the PV matmul — no transposes in the loop):
           S^T_h[j,i] = k_j . q_i  (1/sqrt(dh) folded into wq/bq on host)
           P^T = exp(S^T) on ScalarE, exp evaluated only on the causally
           valid trapezoid (fully-masked left columns of diagonal tiles
           are skipped in scores, exp AND PV — no memsets), triangle band
           masked on DVE; both heads share one 2-bank PSUM tile so exp
           and mask are single wide ops. O^T accumulated in PSUM with
           lhsT = [V_h | ones | pad]; PSUM row 64 = softmax denominator
           for free. Normalize by 1/denominator: DVE fast reciprocal,
           partition-broadcast via a zero-stride DMA through a DRAM
           bounce, deferred into the next block's compute window. PV runs
           two steps behind exp (software pipeline); accumulator banks
           are released early via an SBUF copy.
  phase 3: token ownership is interleaved in 128-token strips (strip s of
           a batch belongs to core s%8) so the AllToAll splits unevenly:
           a2a#0 carries batch 0 + the first half of batch 1 (384
           tokens/core) and fires with ~25us of attention still to run —
           fully hidden; a2a#1 carries only the last two blocks (128
           tokens/core, 0.25MB) and its latency hides under dense2 on the
           a2a#0 data. dense2 runs in fp16 straight off the recv slabs;
           b2' = b1v @ W2 + b2 is added on the host after gathering.
Output: core c returns rows [b0 strip c | b0 strip c+8 | b1 strip c |
b1 strip c+8], 128 tokens each.
"""

import sys

if "/opt/trn_rl_repo" not in sys.path:
    sys.path.insert(0, "/opt/trn_rl_repo")

import numpy as np

import concourse.bass as bass
import concourse.mybir as mybir
import concourse.tile as tile
from concourse import bacc
from concourse.bass_utils import run_bass_kernel_spmd

F32 = mybir.dt.float32
F32R = mybir.dt.float32r
F16 = mybir.dt.float16
AF = mybir.ActivationFunctionType

B, T, D = 2, 2048, 1024
NHEADS, DH = 16, 64
NCORE = 8
TT = B * T            # 4096 global token rows
NCHUNK = 8            # 512-token chunks
NTILE = 32            # 128-token tiles


def build_nc():
    nc = bacc.Bacc(
        "TRN2",
        target_bir_lowering=False,
        debug=False,
        enable_asserts=True,
        num_devices=NCORE,
    )
    # ---- DRAM I/O (per core) ----
    xT_d = nc.dram_tensor("xT", [D, TT], F16, kind="ExternalInput")
    wk_d = nc.dram_tensor("wk", [128, 8, 128], F16, kind="ExternalInput")
    wq_d = nc.dram_tensor("wq", [128, 8, 128], F16, kind="ExternalInput")
    wv_d = nc.dram_tensor("wv", [128, 8, 128], F16, kind="ExternalInput")
    bk_d = nc.dram_tensor("bk", [128, 1], F32, kind="ExternalInput")
    bq_d = nc.dram_tensor("bq", [128, 1], F32, kind="ExternalInput")
    w2_d = nc.dram_tensor("w2", [128, 8, D], F16, kind="ExternalInput")
    masks_d = nc.dram_tensor("masks", [128, 256], F16, kind="ExternalInput")
    ones_d = nc.dram_tensor("ones", [1, 128], F32, kind="ExternalInput")
    ident_d = nc.dram_tensor("ident", [128, 128], F32R, kind="ExternalInput")
    out_d = nc.dram_tensor("out", [512, D], F32, kind="ExternalOutput")

    with tile.TileContext(nc) as tc, nc.allow_low_precision(reason="fp16 pipeline"):
        with (
            tc.tile_pool(name="const", bufs=1) as const,
            tc.tile_pool(name="kq", bufs=1) as kqp,
            tc.tile_pool(name="vp", bufs=1) as vp,
            tc.tile_pool(name="dram", bufs=1, space="DRAM") as dram,
        ):
            # ---- PE warmup: keep the systolic array busy from t=0 so the
            # HAM clock gate opens (K=8/8) before the first projection MM
            # and never re-throttles during the DMA ramp. Junk matmuls on a
            # zeroed tile; no DMA dependencies.
            with (
                tc.tile_pool(name="warm", bufs=1) as warm,
                tc.tile_pool(name="wps", bufs=1, space="PSUM") as wps,
            ):
                wsb = warm.tile([128, 128], F16)
                nc.vector.memset(wsb[:], 0.0)
                wp = wps.tile([128, 64], F32)
                for _ in range(96):
                    nc.tensor.matmul(wp[:], lhsT=wsb[:], rhs=wsb[:, 0:64],
                                     start=True, stop=True)

            # ---- constants (bulky w2/masks DMAs are emitted after phase 1
            # so they don't compete with the xT stream at kernel start) ----
            masks_sb = const.tile([128, 256], F16)
            onesf_sb = const.tile([1, 128], F32)
            ident_sb = const.tile([128, 128], F32R)
            w2_sb = const.tile([128, 8, D], F16)
            bk_sb = const.tile([128, 1], F32)
            bq_sb = const.tile([128, 1], F32)

            # ---- persistent activations ----
            # k/q channel-major in fp16: the score matmuls contract K=64
            # per head at full rate (fp16 is 1 cyc/row at any K, unlike
            # fp32r), and the two heads run concurrently in separate PE
            # row groups (tile_position derived from base partition 0/64)
            kT_sb = kqp.tile([128, TT], F16)     # rows 0-63 h0, 64-127 h1
            qT_sb = kqp.tile([128, TT], F16)
            # v token-major fp16, per 128-token tile: cols 0-63 V_h0,
            # 64 ones, 65-128 V_h1, 129 ones, 130-192 junk; PV lhsT is
            # padded to 128 cols ([65h, 65h+128)) — M=128 streams faster
            v_sb = vp.tile([128, NTILE, 193], F16)
            nc.gpsimd.memset(v_sb[:], 1.0)

            # ---- phase 1: projections ----
            with (
                tc.tile_pool(name="wslice", bufs=1) as wsl,
                tc.tile_pool(name="xin", bufs=3) as xin,
                tc.tile_pool(name="vtc", bufs=2) as vtc,
                tc.tile_pool(name="ps1", bufs=2, space="PSUM") as ps1,
                tc.tile_pool(name="pst", bufs=2, space="PSUM") as pst,
            ):
                wk_sb = wsl.tile([128, 8, 128], F16)
                wq_sb = wsl.tile([128, 8, 128], F16)
                wv_sb = wsl.tile([128, 8, 128], F16)
                nc.sync.dma_start(wk_sb[:], wk_d[:])
                nc.sync.dma_start(wq_sb[:], wq_d[:])
                nc.sync.dma_start(wv_sb[:], wv_d[:])
                nc.sync.dma_start(bk_sb[:], bk_d[:])
                nc.sync.dma_start(bq_sb[:], bq_d[:])
                nc.sync.dma_start(ident_sb[:], ident_d[:])

                xT_r = xT_d.ap().rearrange("(a p) t -> p a t", p=128)

                def emit_transposes(vt_c, i8):
                    for t4 in range(4):
                        ps_tr = pst.tile([128, 128], F32R, tag="tp")
                        nc.tensor.transpose(ps_tr[:], vt_c[:, bass.ts(t4, 128)],
                                            ident_sb[:])
                        vi = i8 * 4 + t4
                        nc.vector.tensor_copy(v_sb[:, vi, 0:64], ps_tr[:, 0:64])
                        nc.vector.tensor_copy(v_sb[:, vi, 65:129], ps_tr[:, 64:128])

                pending_vt = None  # transpose chunk i8-1 during chunk i8's MMs
                for i8 in range(NCHUNK):
                    tsl = bass.ts(i8, 512)
                    # chunk 0 lands one 128-channel slice at a time so the
                    # first matmul starts as early as possible; later
                    # chunks use two bulk DMAs (fewer descriptors streams
                    # faster in steady state)
                    xta = xin.tile([128, 4, 512], F16, tag="xta")
                    xtb = xin.tile([128, 4, 512], F16, tag="xtb")
                    if i8 == 0:
                        for a in range(4):
                            nc.sync.dma_start(xta[:, a, :], xT_r[:, a, tsl])
                        nc.sync.dma_start(xtb[:], xT_r[:, 4:8, tsl])
                    else:
                        nc.sync.dma_start(xta[:], xT_r[:, 0:4, tsl])
                        nc.sync.dma_start(xtb[:], xT_r[:, 4:8, tsl])

                    def xt(a):
                        return xta[:, a, :] if a < 4 else xtb[:, a - 4, :]

                    # kT
                    psk = ps1.tile([128, 512], F32, tag="proj")
                    for a in range(8):
                        nc.tensor.matmul(psk[:], lhsT=wk_sb[:, a, :], rhs=xt(a),
                                         start=(a == 0), stop=(a == 7))
                    nc.scalar.activation(kT_sb[:, tsl], psk[:], AF.Identity,
                                         bias=bk_sb[:], scale=1.0)
                    # qT (wq/bq pre-scaled by 1/sqrt(dh) on host)
                    psq = ps1.tile([128, 512], F32, tag="proj")
                    for a in range(8):
                        nc.tensor.matmul(psq[:], lhsT=wq_sb[:, a, :], rhs=xt(a),
                                         start=(a == 0), stop=(a == 7))
                    nc.scalar.activation(qT_sb[:, tsl], psq[:], AF.Identity,
                                         bias=bq_sb[:], scale=1.0)
                    # vT (channel-major) then PE-transpose to token-major
                    psv = ps1.tile([128, 512], F32, tag="proj")
                    for a in range(8):
                        nc.tensor.matmul(psv[:], lhsT=wv_sb[:, a, :], rhs=xt(a),
                                         start=(a == 0), stop=(a == 7))
                    vt_c = vtc.tile([128, 512], F32R, tag="vt")
                    nc.scalar.activation(vt_c[:], psv[:], AF.Identity)
                    if pending_vt is not None:
                        emit_transposes(*pending_vt)
                    pending_vt = (vt_c, i8)
                emit_transposes(*pending_vt)

            # bulky constants for later phases — DMA'd while phase 1 computes
            nc.sync.dma_start(masks_sb[:], masks_d[:])
            nc.sync.dma_start(onesf_sb[:], ones_d[:])
            nc.sync.dma_start(w2_sb[:], w2_d[:])

            # Strip-interleaved AllToAll: strip s (128 tokens) of batch b
            # belongs to core s%8. The exchange is split into four even
            # 0.25MB collectives, one per pair of attention blocks, fired
            # as soon as that pair is normalized — the Comms engine runs
            # them back-to-back ~12GB/s, so #0-#2 hide under attention and
            # only #3 (last two blocks) pokes out, hidden under dense2 on
            # the earlier data. Collective i carries strip (i%2)*8 + r of
            # batch i//2 to peer r.
            a2a_send = [dram.tile([8, 128, 128], F16, name=f"a2a_send{i}")
                        for i in range(4)]
            a2a_recv = [dram.tile([8, 128, 128], F16, name=f"a2a_recv{i}")
                        for i in range(4)]
            norm_dram = dram.tile([8, 2, 512], F32)  # 1/denominator rows

            def emit_a2a(i):
                nc.gpsimd.collective_compute(
                    "AllToAll",
                    mybir.AluOpType.bypass,
                    replica_groups=[list(range(NCORE))],
                    ins=[a2a_send[i].opt()],
                    outs=[a2a_recv[i].opt()],
                )

            # Dummy collective fired during phase 1: the first collective
            # pays a ~12us Comms-engine warmup; burn it on junk data while
            # the projections run.
            dmy_send = dram.tile([8, 128, 8], F16)
            dmy_recv = dram.tile([8, 128, 8], F16)
            nc.gpsimd.collective_compute(
                "AllToAll",
                mybir.AluOpType.bypass,
                replica_groups=[list(range(NCORE))],
                ins=[dmy_send.opt()],
                outs=[dmy_recv.opt()],
            )

            # ---- phase 2: attention ----
            with (
                tc.tile_pool(name="pp", bufs=4) as pp,
                tc.tile_pool(name="otp", bufs=2) as otp,
                tc.tile_pool(name="bcp", bufs=2) as bcp,
                tc.tile_pool(name="rcp", bufs=2) as rcp,
                tc.tile_pool(name="pss", bufs=2, space="PSUM") as pss,
                tc.tile_pool(name="pso", bufs=4, space="PSUM") as pso,
            ):
                def emit_pv(p_pair, b, kj, po, nkj, dp):
                    # diagonal tiles (dp>0): q-columns < dp*128 are fully
                    # masked — the matmul just skips them (kj==0 always
                    # writes the full width, so every column has its
                    # start=True write)
                    off = dp * 128 if dp > 0 else 0
                    for h in range(2):
                        nc.tensor.matmul(
                            po[h][:, off:512],
                            lhsT=v_sb[:, b * 16 + kj, 65 * h:65 * h + 128],
                            rhs=p_pair[h][:, off:512],
                            start=(kj == 0), stop=(kj == nkj - 1),
                            skip_group_check=True,
                        )

                def emit_norm(po, blk, last=False):
                    # normalize O^T rows 0-63 by 1/denominator (row 64),
                    # then broadcast the row across 64 partitions with a
                    # zero-stride DMA through a DRAM bounce — the PE is not
                    # involved, and the whole chain is deferred into the
                    # NEXT block's compute. For the last two blocks a K=1
                    # broadcast matmul replaces the DMA round-trip: its
                    # latency is lower and it keeps the chain to the final
                    # collective trigger short (the PE cost is ~1us).
                    b, qi = blk // 4, blk % 4
                    last = last or blk >= 6
                    for h in range(2):
                        # 1/denominator via the fast DVE approx (needs an
                        # SBUF source); keeping this off ScalarE avoids
                        # Exp<->Ln activation-table reloads (1.3us each)
                        dn = rcp.tile([1, 512], F32, tag="dn")
                        nc.vector.tensor_copy(dn[:], po[h][64:65, :])
                        rc = rcp.tile([1, 512], F32, tag="rc")
                        nc.vector.reciprocal_approx_fast(rc[:], dn[:])
                        # copy O^T out of PSUM right away so the po bank is
                        # released before the DMA broadcast round-trip
                        ou = otp.tile([64, 512], F32, tag="ou")
                        nc.vector.tensor_copy(ou[:], po[h][0:64, :])
                        bc = bcp.tile([64, 512], F32, tag="bcs")
                        if last:
                            pb = pso.tile([128, 512], F32, tag="o")
                            nc.tensor.matmul(pb[:], lhsT=onesf_sb[:, :], rhs=rc[:],
                                             start=True, stop=True)
                            nc.vector.tensor_copy(bc[:], pb[0:64, :])
                        else:
                            nc.sync.dma_start(norm_dram[blk, h], rc[:])
                            row = norm_dram[blk, h]
                            nc.sync.dma_start(
                                bc[:],
                                bass.AP(row.tensor, row.offset, [[0, 64], [1, 512]]))
                        ot = otp.tile([64, 512], F16, tag="ot")
                        nc.vector.tensor_mul(ot[:], ou[:], bc[:])
                        # scatter the four 128-token strips to their
                        # owners' slots
                        for si in range(4):
                            s = 4 * qi + si
                            own = s % 8
                            ci = b * 2 + s // 8
                            src = ot[:, si * 128:si * 128 + 128]
                            hsl = slice(64 * h, 64 * h + 64)
                            nc.sync.dma_start(
                                a2a_send[ci][own, hsl, 0:128], src)

                pending_norm = None  # previous block's (po, blk)

                def flush_norm(last=False):
                    nonlocal pending_norm
                    if pending_norm is None:
                        return
                    po, blk = pending_norm
                    pending_norm = None
                    emit_norm(po, blk, last=last)
                    if blk in (1, 3, 5):
                        emit_a2a(blk // 2)  # block pair complete

                for b in range(B):
                    for qi in range(4):
                        qoff = b * T + qi * 512
                        nkj = 4 * qi + 4
                        po0 = pso.tile([128, 512], F32, tag="o")
                        po1 = pso.tile([128, 512], F32, tag="o")
                        po = [po0, po1]
                        pv_queue = []  # PV runs two kj behind S/exp
                        for kj in range(nkj):
                            koff = b * T + kj * 128
                            dp = kj - 4 * qi  # >=0: diagonal tile index
                            off = dp * 128 if dp > 0 else 0
                            # both heads' score tiles in one 2-bank PSUM
                            # tile so exp and mask are single wide ops; the
                            # two K=64 fp16 matmuls sit in different PE row
                            # groups and execute concurrently
                            ss = pss.tile([128, 1024], F32, tag="s")
                            for h in range(2):
                                nc.tensor.matmul(
                                    ss[:, 512 * h + off:512 * h + 512],
                                    lhsT=kT_sb[64 * h:64 * h + 64, koff:koff + 128],
                                    rhs=qT_sb[64 * h:64 * h + 64,
                                              qoff + off:qoff + 512],
                                    start=True, stop=True,
                                )
                            p = pp.tile([128, 1024], F16, tag="p")
                            if dp <= 0:
                                nc.scalar.activation(p[:], ss[:], AF.Exp)
                            else:
                                # causal: cols < dp*128 are fully masked —
                                # exp only the valid right part; the masked
                                # columns are never read (PV is sliced)
                                p_r = p[:].rearrange("q (h c) -> q h c", h=2)
                                s_r = ss[:].rearrange("q (h c) -> q h c", h=2)
                                nc.scalar.activation(p_r[:, :, off:512],
                                                     s_r[:, :, off:512], AF.Exp)
                            if dp >= 0:
                                # triangle band at the causal boundary
                                p_r = p[:].rearrange("q (h c) -> q h c", h=2)
                                m_r = masks_sb[:].rearrange("q (h c) -> q h c", h=2)
                                nc.vector.tensor_mul(
                                    p_r[:, :, off:off + 128],
                                    p_r[:, :, off:off + 128], m_r[:])
                            p_pair = [p[:, 0:512], p[:, 512:1024]]
                            pv_queue.append((p_pair, b, kj, po, nkj, dp))
                            if len(pv_queue) > 2:
                                emit_pv(*pv_queue.pop(0))
                            if kj == 3:
                                flush_norm()
                        for ppv in pv_queue:
                            emit_pv(*ppv)
                        flush_norm()
                        pending_norm = (po, b * 4 + qi)
                flush_norm(last=True)
                emit_a2a(3)

            # ---- phase 3: dense2 over this core's four 128-token strips.
            # Groups 0-2 come from a2a#0 (done before attention ends) and
            # run while a2a#1 is still in flight; group 3 (from a2a#1) is
            # the only exposed piece.
            with (
                tc.tile_pool(name="osb", bufs=1) as osbp,
                tc.tile_pool(name="obp", bufs=3) as obp,
                tc.tile_pool(name="psd", bufs=4, space="PSUM") as psd,
            ):
                slabs = [[], [], [], []]
                for i in range(4):
                    for a in range(8):
                        sl = osbp.tile([128, 128], F16, name=f"s{i}_{a}")
                        nc.sync.dma_start(sl[:], a2a_recv[i][a])
                        slabs[i].append(sl)
                for g in range(4):
                    pd0 = psd.tile([128, 512], F32, tag="d")
                    pd1 = psd.tile([128, 512], F32, tag="d")
                    pd = [pd0, pd1]
                    for a in range(8):
                        lhsT = slabs[g][a][:, 0:128]
                        # both n-halves back-to-back: shared lhsT load
                        for n2 in range(2):
                            nc.tensor.matmul(
                                pd[n2][:],
                                lhsT=lhsT,
                                rhs=w2_sb[:, a, bass.ts(n2, 512)],
                                start=(a == 0), stop=(a == 7),
                                skip_group_check=True,
                            )
                    for n2 in range(2):
                        nsl = bass.ts(n2, 512)
                        ob = obp.tile([128, 512], F32, tag="ob")
                        nc.vector.tensor_copy(ob[:], pd[n2][:])
                        nc.sync.dma_start(
                            out_d[g * 128:g * 128 + 128, nsl], ob[:])

    nc.compile()
    return nc


_NC_CACHE = {}


def get_nc():
    if "nc" not in _NC_CACHE:
        _NC_CACHE["nc"] = build_nc()
    return _NC_CACHE["nc"]


def make_in_maps(x, W1, b1, W2, b2):
    x = np.asarray(x, dtype=np.float32)
    W1 = np.asarray(W1, dtype=np.float32)
    b1 = np.asarray(b1, dtype=np.float32)
    W2 = np.asarray(W2, dtype=np.float32)
    b2 = np.asarray(b2, dtype=np.float32)

    scale = np.float32(1.0 / np.sqrt(DH))
    xT = np.ascontiguousarray(x.reshape(TT, D).T)  # [D, TT]
    Wk, Wq, Wv = W1[:, :D], W1[:, D:2 * D], W1[:, 2 * D:]
    bk, bq, bv = b1[:D], b1[D:2 * D], b1[2 * D:]

    # causal triangle band mask [128k x 128q], duplicated for both heads
    j = np.arange(128)[:, None]
    il = np.arange(128)[None, :]
    masks = np.tile((il >= j).astype(np.float32), (1, 2))

    ones = np.ones((1, 128), np.float32)
    ident = np.eye(128, dtype=np.float32)

    def stack(w):  # [1024, m] -> [128, 8, m] with [p, a, :] = w[a*128+p]
        return np.ascontiguousarray(
            w.reshape(8, 128, -1).transpose(1, 0, 2))

    w2s = stack(W2).astype(np.float16)
    xT16 = xT.astype(np.float16)
    masks16 = masks.astype(np.float16)
    in_maps = []
    for c in range(NCORE):
        csl = slice(c * 128, (c + 1) * 128)
        in_maps.append({
            "xT": xT16,
            "wk": stack(Wk[:, csl]).astype(np.float16),
            "wq": stack(Wq[:, csl] * scale).astype(np.float16),
            "wv": stack(Wv[:, csl]).astype(np.float16),
            "bk": bk[csl].reshape(128, 1).copy(),
            "bq": (bq[csl] * scale).reshape(128, 1).copy(),
            "w2": w2s,
            "masks": masks16,
            "ones": ones,
            "ident": ident,
        })
    return in_maps


def assemble(results, b2p):
    out = np.empty((B, T, D), dtype=np.float32)
    for c in range(NCORE):
        r = results[c]["out"]
        out[0, c * 128:(c + 1) * 128, :] = r[0:128]
        out[0, (c + 8) * 128:(c + 9) * 128, :] = r[128:256]
        out[1, c * 128:(c + 1) * 128, :] = r[256:384]
        out[1, (c + 8) * 128:(c + 9) * 128, :] = r[384:512]
    out += b2p
    return out


def kernel(x, W1, b1, W2, b2, _trace=False):
    nc = get_nc()
    in_maps = make_in_maps(x, W1, b1, W2, b2)
    W2f = np.asarray(W2, dtype=np.float32)
    b2p = (np.asarray(b1, np.float32)[2 * D:] @ W2f
           + np.asarray(b2, np.float32)).reshape(1, 1, D)
    kw = {"trace_cores": list(range(NCORE))} if _trace else {}
    res = run_bass_kernel_spmd(
        nc, in_maps, core_ids=list(range(NCORE)), trace=_trace, **kw)
    out = assemble(res.results, b2p)
    if _trace:
        return out, res
    return out
